# revision 12
# baseline (speedup 1.0000x reference)
"""Trainium2 Bass kernel for nn_AttentionBlock (GroupNorm -> MHA -> proj + residual).

Contract: kernel(**inputs) takes the FULL unsharded inputs (as produced by
setup_inputs) and returns the FULL output [8, 512, 32, 32] float32.

Sharding: pure data-parallel over batch B=8 across the 8 NeuronCores; each core
processes one batch element end-to-end (no collectives needed).

Per-core layout / algorithm (B=1, C=512, N=H*W=1024, heads=8, head_dim=64):
  - Head: x arrives as 8 half-tile DMAs spread over 4 engine queues; all small
    tensors ride ONE packed [128,28] DMA; dummy matmuls on a memset tile keep
    the PE HAM-warm (2.4 GHz) through the DMA window so the real stream never
    pays the 1.2 GHz cold clock.
  - GroupNorm: per-channel bn_stats chase the x half-DMAs; group-combine +
    broadcast via tiny PE matmuls split r0..2 / r3 so h0..2 finish before x3
    even lands; normalize splits across ScalarE/VectorE; h also cast to fp8
    on GpSimd for the v/proj path.
  - qkv 1x1-conv as matmuls with host-pre-transposed weights; q scale folded.
    q/k stay bf16 (fp8 there fails the error budget); wv/pw are fp8 e4m3 and
    their matmuls run DoubleRow over kc-tile pairs (2x fewer PE passes).
  - Attention (half-major order: token-half outer, head-pair inner, so the
    first half's proj + output DMA drain mid-stream). S^T per head via
    lhsT=k row-tiled pairs (hi=0/1 concurrent in the 128x128 array).
    exp(S-5) on ScalarE straight to fp8 e4m3 (softmax is shift-invariant;
    shift keeps e^ under the 240 fp8 max). A@V runs DoubleRow over m-tile
    pairs with lhsT = [ones|v] fp8, so denominators come out broadcast at
    PSUM partitions 0:64 for the custom-DVE fast reciprocal. A subset of
    pairs offloads exp to VectorE via the Schraudolph int16 trick (bf16 E,
    plain matmuls) to balance ScalarE vs PE.
  - proj matmuls DoubleRow over kc pairs from fp8 O; x+pb precomputed on
    GpSimd; residual add + output DMA per token-half.
"""

import numpy as np
import ml_dtypes

import concourse.bass as bass
import concourse.tile as tile
from concourse import bacc, mybir
from concourse.bass_utils import run_bass_kernel_spmd

FP32 = mybir.dt.float32
BF16 = mybir.dt.bfloat16
F8 = mybir.dt.float8e4
I16 = mybir.dt.int16
AF = mybir.ActivationFunctionType
OP = mybir.AluOpType
DR = mybir.MatmulPerfMode.DoubleRow

P = 128      # SBUF partitions
C = 512      # channels
NT = 1024    # spatial tokens (32*32)
CT = C // P  # channel tiles = 4
MT = NT // P # m (key) tiles = 8
NH = 8       # heads
HD = 64      # head dim
NCORES = 8
GSZ = 16     # channels per group (512/32)

USE_FP8 = True
SHIFT = 5.0          # exp(S - SHIFT): keeps e^ <= ~240 (fp8 max) w/ margin
WARM_MMS = 26        # dummy matmuls to hold the PE HAM-warm through DMA-in
# global pair indices (0..31) whose exp runs on VectorE (Schraudolph int16).
SCH_PAIRS = frozenset()
A_SCH = 128.0 / float(np.log(2.0))
B_SCH = 127.0 * 128.0 - 5.5


def _emit(tc: "tile.TileContext", io: dict):
    nc = tc.nc
    x, wq, wk, wv, pw = io["x"], io["wq"], io["wk"], io["wv"], io["pw"]
    smalls, imat = io["smalls"], io["imat"]
    out = io["out"]
    wdt = F8 if USE_FP8 else BF16

    import contextlib
    ctx = contextlib.ExitStack()
    with ctx:
        pers = ctx.enter_context(tc.tile_pool(name="pers", bufs=1))
        sm = ctx.enter_context(tc.tile_pool(name="small", bufs=1))

        # ---------------- input DMAs ----------------
        # x half-tiles spread over 4 queues so GN stats chase arrivals;
        # weights next on the queues that are otherwise idle; smalls packed.
        x_r = x.rearrange("(r p) n -> p r n", p=P)
        x_sb = pers.tile([P, CT, NT], FP32, tag="x")
        dummy = pers.tile([P, 512], BF16, tag="dummy")
        nc.gpsimd.memset(dummy, 0.5)  # first: unblocks the PE warm spam
        # x tiles 0-2 one per DMA queue; tile 3 (GN's critical tail) split
        # in halves across two queues so its stats start earliest
        nc.sync.dma_start(x_sb[:, 0, :], x_r[:, 0, :])
        nc.gpsimd.dma_start(x_sb[:, 1, :], x_r[:, 1, :])
        nc.scalar.dma_start(x_sb[:, 2, :], x_r[:, 2, :])
        nc.sync.dma_start(x_sb[:, 3, 0:512], x_r[:, 3, 0:512])
        nc.gpsimd.dma_start(x_sb[:, 3, 512:1024], x_r[:, 3, 512:1024])
        sm_sb = pers.tile([P, 28], FP32, tag="smalls")
        nc.scalar.dma_start(sm_sb, smalls)
        imat_sb = pers.tile([NH, P], FP32, tag="imat")
        nc.scalar.dma_start(imat_sb, imat)
        gg_sb, gb_sb = sm_sb[:, 0:4], sm_sb[:, 4:8]
        bq_sb, bk_sb = sm_sb[:, 8:12], sm_sb[:, 12:16]
        pb_sb, amat_sb = sm_sb[:, 16:20], sm_sb[:, 20:28]
        wq_sb = pers.tile([P, CT, C], BF16, tag="wq")
        nc.scalar.dma_start(wq_sb, wq.rearrange("(k p) o -> p k o", p=P))
        wk_sb = pers.tile([P, CT, C], BF16, tag="wk")
        nc.sync.dma_start(wk_sb, wk.rearrange("(k p) o -> p k o", p=P))
        wv_sb = pers.tile([P, CT, C], wdt, tag="wv")
        nc.gpsimd.dma_start(wv_sb, wv.rearrange("(k p) o -> p k o", p=P))
        pw_sb = pers.tile([P, CT, C], wdt, tag="pw")
        nc.sync.dma_start(pw_sb, pw.rearrange("(k p) o -> p k o", p=P))
        # preload the exp activation table while DMAs are in flight
        warm_sb = pers.tile([1, 1], FP32, tag="actwarm")
        nc.vector.memset(warm_sb, 0.0)
        nc.scalar.activation(warm_sb, warm_sb, AF.Exp)
        # per-partition -SHIFT bias column for the shifted exp
        shift_sb = pers.tile([P, 1], FP32, tag="shift")
        nc.vector.memset(shift_sb, -SHIFT)

        # v^T with interleaved ones columns: per head 128 cols = [ones(64) | v(64)]
        vT_sb = pers.tile([P, MT, NH * 128], wdt, tag="vT")
        nc.gpsimd.memset(
            vT_sb.rearrange("p t (h c) -> p t h c", c=128)[:, :, :, 0:HD], 1.0)

        h_sb = pers.tile([P, CT, NT], BF16, tag="h")
        if USE_FP8:
            h8_sb = pers.tile([P, CT, NT], wdt, tag="h8", name="h8")
        else:
            h8_sb = h_sb
        q_sb = pers.tile([P, CT, NT], BF16, tag="q")
        k_sb = pers.tile([P, CT, NT], BF16, tag="k")
        O_sb = pers.tile([P, CT, NT], wdt, tag="O")
        xpb_sb = pers.tile([P, CT, NT], FP32, tag="xpb")

        # ---------------- PE warm spam ----------------
        # The HAM clock gate defaults to 1.2 GHz and only opens to 2.4 GHz
        # after ~3.4us of sustained PE activity. Burn idle DMA-wait time on
        # dummy matmuls so the real stream starts (and stays) warm.
        with tc.tile_pool(name="warmps", bufs=1, space="PSUM") as wps:
            wt = wps.tile([P, 512], FP32, tag="warm")
            for _ in range(WARM_MMS):
                nc.tensor.matmul(wt, dummy[:, 0:128], dummy,
                                 start=True, stop=True)

        # ---------------- GroupNorm (per-tile pipelined) ----------------
        # groups are 16 channels wide so every group lives inside one
        # 128-channel tile; tiles are fully independent. r0..2 finish while
        # x3 is still in flight; r3 runs the short critical chain alone.
        with nc.named_scope("gn"), \
             tc.tile_pool(name="gnps", bufs=1, space="PSUM") as gnps:
            stats = sm.tile([P, CT, 2, 6], FP32, tag="bnst")
            mv = sm.tile([P, CT, 2], FP32, tag="gnmv")
            st2 = sm.tile([P, CT, 2], FP32, tag="gnst2")
            for r in range(CT):
                nc.vector.bn_stats(stats[:, r, 0, :], x_sb[:, r, 0:512])
                nc.vector.bn_stats(stats[:, r, 1, :], x_sb[:, r, 512:1024])
                nc.vector.bn_aggr(mv[:, r, :], stats[:, r])
                nc.vector.tensor_copy(st2[:, r, 0:1], mv[:, r, 0:1])
                nc.vector.tensor_tensor(st2[:, r, 1:2], mv[:, r, 0:1],
                                        mv[:, r, 0:1], OP.mult)
                nc.vector.tensor_tensor(st2[:, r, 1:2], st2[:, r, 1:2],
                                        mv[:, r, 1:2], OP.add)
            G_ps = gnps.tile([NH, CT, 2], FP32, tag="gps")
            MR_ps = gnps.tile([P, CT, 2], FP32, tag="mrps")
            st_all = sm.tile([NH, CT, 2], FP32, tag="gnsta")
            var_all = sm.tile([NH, CT], FP32, tag="gnvar")
            y_t = sm.tile([NH, CT, 2], FP32, tag="gnyt")
            mr = sm.tile([P, CT, 2], FP32, tag="gnmr")
            ab = sm.tile([P, CT, 2], FP32, tag="gnab")

            def gn_finish(lo, hi):
                sl = slice(lo, hi)
                # per-group (mean, E[x^2]) for tiles [lo,hi) in one matmul
                nc.tensor.matmul(G_ps[:, sl, :], amat_sb, st2[:, sl, :],
                                 start=True, stop=True)
                nc.vector.tensor_copy(st_all[:, sl, :], G_ps[:, sl, :])
                nc.vector.tensor_tensor(var_all[:, sl, None],
                                        st_all[:, sl, 0:1],
                                        st_all[:, sl, 0:1], OP.mult)
                nc.vector.tensor_tensor(var_all[:, sl, None],
                                        st_all[:, sl, 1:2],
                                        var_all[:, sl, None], OP.subtract)
                nc.vector.tensor_scalar(var_all[:, sl], var_all[:, sl],
                                        1e-5, None, OP.add)
                y = y_t[:, sl, 0:1]
                t = y_t[:, sl, 1:2]
                va = var_all[:, sl, None]
                nc.vector.reciprocal_approx_fast(y, va)
                for it in range(2):
                    nc.vector.tensor_tensor(t, y, y, OP.mult)
                    nc.vector.tensor_tensor(t, t, va, OP.mult)
                    nc.vector.tensor_scalar(t, t, -0.5, 1.5, OP.mult, OP.add)
                    if it < 1:
                        nc.vector.tensor_tensor(y, y, t, OP.mult)
                    else:
                        nc.vector.tensor_tensor(st_all[:, sl, 1:2], y, t,
                                                OP.mult)
                # broadcast (mean, rstd) back to channels
                nc.tensor.matmul(MR_ps[:, sl, :], imat_sb, st_all[:, sl, :],
                                 start=True, stop=True)
                nc.vector.tensor_copy(mr[:, sl, :], MR_ps[:, sl, :])
                nc.vector.tensor_tensor(ab[:, sl, 0:1], mr[:, sl, 1:2],
                                        gg_sb[:, sl, None], OP.mult)
                nc.vector.tensor_tensor(ab[:, sl, 1:2], mr[:, sl, 0:1],
                                        ab[:, sl, 0:1], OP.mult)
                nc.vector.tensor_tensor(ab[:, sl, 1:2], gb_sb[:, sl, None],
                                        ab[:, sl, 1:2], OP.subtract)
                for r in range(lo, hi):
                    if r < CT - 1:
                        nc.scalar.activation(h_sb[:, r, :], x_sb[:, r, :],
                                             AF.Identity, bias=ab[:, r, 1:2],
                                             scale=ab[:, r, 0:1])
                    else:
                        # last tile: split halves across ScalarE/VectorE to
                        # shorten the critical path into the first qkv matmul
                        nc.scalar.activation(h_sb[:, r, 0:512],
                                             x_sb[:, r, 0:512],
                                             AF.Identity, bias=ab[:, r, 1:2],
                                             scale=ab[:, r, 0:1])
                        nc.vector.tensor_scalar(h_sb[:, r, 512:1024],
                                                x_sb[:, r, 512:1024],
                                                ab[:, r, 0:1], ab[:, r, 1:2],
                                                OP.mult, OP.add)
                    if USE_FP8:
                        nc.gpsimd.tensor_copy(h8_sb[:, r, :], h_sb[:, r, :])

            gn_finish(0, CT - 1)
            gn_finish(CT - 1, CT)

        # ------------- qkv + attention (interleaved on PE) -------------
        # PSUM (8 banks): S chunks [128,2,512] x2 tags (4) + O pair-half
        # [128,2,512] (2) + background qkv/proj accumulators [128,512] x2 (2).
        with nc.named_scope("qkv_attn"), \
             tc.tile_pool(name="bgps", bufs=2, space="PSUM") as bgps, \
             tc.tile_pool(name="spool", bufs=1, space="PSUM") as spool, \
             tc.tile_pool(name="opool", bufs=1, space="PSUM") as opool, \
             tc.tile_pool(name="epool", bufs=3) as epool, \
             tc.tile_pool(name="rpool", bufs=2) as rpool, \
             tc.tile_pool(name="outp", bufs=4) as outp:

            def qk_task(dst, w_sb, b_sb, r, half, on_act=False):
                ps = bgps.tile([P, 512], FP32, tag="bgps",
                               name=f"qk_{r}_{half}_{w_sb.name}")
                for kc in range(CT):
                    nc.tensor.matmul(
                        ps, w_sb[:, kc, P * r:P * r + P],
                        h_sb[:, kc, 512 * half:512 * half + 512],
                        start=(kc == 0), stop=(kc == CT - 1))
                dsl = dst[:, r, 512 * half:512 * half + 512]
                if on_act:
                    nc.scalar.activation(dsl, ps, AF.Identity,
                                         bias=b_sb[:, r:r + 1], scale=1.0)
                else:
                    nc.vector.tensor_scalar(dsl, ps, b_sb[:, r:r + 1],
                                            None, OP.add)

            def vt_task(t):
                ps = bgps.tile([P, 512], FP32, tag="bgps", name=f"vt{t}")
                if USE_FP8:
                    for s in range(CT // 2):
                        nc.tensor.matmul(
                            ps, h8_sb[:, 2 * s:2 * s + 2, P * t:P * t + P],
                            wv_sb[:, 2 * s:2 * s + 2, :],
                            start=(s == 0), stop=(s == CT // 2 - 1),
                            perf_mode=DR)
                else:
                    for kc in range(CT):
                        nc.tensor.matmul(ps, h8_sb[:, kc, P * t:P * t + P],
                                         wv_sb[:, kc, :],
                                         start=(kc == 0), stop=(kc == CT - 1))
                nc.vector.tensor_copy(
                    vT_sb[:, t, :].rearrange("p (h c) -> p h c", c=128)[:, :, HD:128],
                    ps.rearrange("p (h c) -> p h c", c=HD))

            def xpb_task(rr):
                nc.gpsimd.tensor_scalar(xpb_sb[:, rr, :], x_sb[:, rr, :],
                                        pb_sb[:, rr:rr + 1], None, OP.add)

            out_r = out.rearrange("(r p) n -> p r n", p=P)

            def proj_mms(ps, r, half, lo_pair):
                hs = 512 * half
                s = lo_pair
                if USE_FP8:
                    nc.tensor.matmul(
                        ps, pw_sb[:, 2 * s:2 * s + 2, P * r:P * r + P],
                        O_sb[:, 2 * s:2 * s + 2, hs:hs + 512],
                        start=True, stop=True, perf_mode=DR)
                else:
                    for kc in (2 * s, 2 * s + 1):
                        nc.tensor.matmul(
                            ps, pw_sb[:, kc, P * r:P * r + P],
                            O_sb[:, kc, hs:hs + 512],
                            start=(kc == 2 * s), stop=(kc == 2 * s + 1))

            def proj_part(r, half):
                # kc pair (0,1): heads 0..3 of this token half + x + pb,
                # accumulated in place into xpb
                hs = 512 * half
                ps = bgps.tile([P, 512], FP32, tag="bgps",
                               name=f"pp{r}_{half}")
                proj_mms(ps, r, half, 0)
                nc.vector.tensor_tensor(xpb_sb[:, r, hs:hs + 512], ps,
                                        xpb_sb[:, r, hs:hs + 512], OP.add)

            def proj_fin(r, half, eng_i=0):
                hs = 512 * half
                ps = bgps.tile([P, 512], FP32, tag="bgps",
                               name=f"pf{r}_{half}")
                proj_mms(ps, r, half, 1)
                o_sb = outp.tile([P, 512], FP32, tag="outsb",
                                 name=f"osb{r}_{half}")
                nc.vector.tensor_tensor(o_sb, ps,
                                        xpb_sb[:, r, hs:hs + 512], OP.add)
                eng = (nc.sync, nc.gpsimd)[eng_i % 2]
                eng.dma_start(out_r[:, r, hs:hs + 512], o_sb)

            # upfront: only what attention tile 0 needs (q0/k0 first halves)
            qk_task(q_sb, wq_sb, bq_sb, 0, 0, on_act=True)
            qk_task(k_sb, wk_sb, bk_sb, 0, 0, on_act=True)

            # drip key k fires right after stream tile T=k+1; keys 7,15,...
            # land just before a head-pair boundary where the PE otherwise
            # stalls on the last exp, so they hold PE-heavy tasks.
            drip = {
                0: [(vt_task, (0,)), (vt_task, (1,))],
                1: [(vt_task, (2,)), (vt_task, (3,))],
                2: [(qk_task, (q_sb, wq_sb, bq_sb, 1, 0))],
                3: [(qk_task, (k_sb, wk_sb, bk_sb, 1, 0))],
                4: [(vt_task, (4,))], 5: [(vt_task, (5,))],
                6: [(vt_task, (6,))], 7: [(vt_task, (7,))],
                10: [(qk_task, (q_sb, wq_sb, bq_sb, 2, 0))],
                11: [(qk_task, (k_sb, wk_sb, bk_sb, 2, 0))],
                12: [(xpb_task, (0,)), (xpb_task, (1,))],
                13: [(xpb_task, (2,)), (xpb_task, (3,))],
                15: [(qk_task, (q_sb, wq_sb, bq_sb, 3, 0))],
                16: [(qk_task, (k_sb, wk_sb, bk_sb, 3, 0))],
                20: [(proj_part, (0, 0))], 21: [(proj_part, (1, 0))],
                22: [(proj_part, (2, 0))], 23: [(proj_part, (3, 0))],
                26: [(qk_task, (q_sb, wq_sb, bq_sb, 0, 1))],
                27: [(qk_task, (k_sb, wk_sb, bk_sb, 0, 1))],
                32: [(proj_fin, (0, 0, 0))], 33: [(proj_fin, (1, 0, 1))],
                34: [(qk_task, (q_sb, wq_sb, bq_sb, 1, 1))],
                35: [(qk_task, (k_sb, wk_sb, bk_sb, 1, 1))],
                36: [(proj_fin, (2, 0, 0))], 37: [(proj_fin, (3, 0, 1))],
                39: [(qk_task, (q_sb, wq_sb, bq_sb, 2, 1))],
                40: [(qk_task, (k_sb, wk_sb, bk_sb, 2, 1))],
                46: [(qk_task, (q_sb, wq_sb, bq_sb, 3, 1))],
                47: [(qk_task, (k_sb, wk_sb, bk_sb, 3, 1))],
                52: [(proj_part, (0, 1))], 53: [(proj_part, (1, 1))],
                54: [(proj_part, (2, 1))], 55: [(proj_part, (3, 1))],
            }

            # ---- the unit stream: half-major, AV in DoubleRow m-tile pairs
            O_cur = [None]

            def emit_av_pair(pr, half, tp, E_sup, is_f8):
                for hi in range(2):
                    h8c = 128 * (2 * pr + hi)
                    if is_f8:
                        nc.tensor.matmul(
                            O_cur[0][:, hi, :],
                            vT_sb[:, 2 * tp:2 * tp + 2, h8c:h8c + 128],
                            E_sup[:, :, hi, :],
                            start=(tp == 0), stop=(tp == 3),
                            perf_mode=DR)
                    else:
                        for tpar in range(2):
                            nc.tensor.matmul(
                                O_cur[0][:, hi, :],
                                vT_sb[:, 2 * tp + tpar, h8c:h8c + 128],
                                E_sup[:, tpar, hi, :],
                                start=(tp == 0 and tpar == 0),
                                stop=(tp == 3 and tpar == 1))

            def emit_epilogue(pr, half, last=False):
                hs = 512 * half
                O_half = O_cur[0]
                Rh = rpool.tile([HD, 2, 512], FP32, tag="rh",
                                name=f"rh{pr}_{half}")
                if last:
                    # split per-hi so the final proj matmuls start sooner
                    for hi in range(2):
                        nc.vector.reciprocal_approx_fast(
                            Rh[:, hi, :], O_half[0:HD, hi, :])
                        nc.vector.tensor_tensor(
                            O_sb[HD * hi:HD * hi + HD, pr, hs:hs + 512],
                            O_half[HD:128, hi, :], Rh[:, hi, :], OP.mult)
                else:
                    nc.vector.reciprocal_approx_fast(Rh, O_half[0:HD, :, :])
                    for hi in range(2):
                        nc.vector.tensor_tensor(
                            O_sb[HD * hi:HD * hi + HD, pr, hs:hs + 512],
                            O_half[HD:128, hi, :], Rh[:, hi, :], OP.mult)

            pend = []  # deferred AV pair flushes: (tile_due, closure)
            T = 0
            fired = 0
            for half in range(2):
                for pr in range(4):
                    O_cur[0] = opool.tile([P, 2, 512], FP32, tag="oh",
                                          name=f"oh{pr}_{half}")
                    for tp in range(4):
                        gpi = (4 * half + pr) * 4 + tp
                        sch = gpi in SCH_PAIRS
                        is_f8 = USE_FP8 and not sch
                        if is_f8:
                            E_sup = epool.tile([P, 2, 2, 512], F8, tag="e8",
                                               name=f"e{gpi}")
                        else:
                            E_i = epool.tile([P, 2, 2, 512], I16, tag="ebf",
                                             name=f"e{gpi}")
                            E_sup = E_i.bitcast(BF16)
                        for tpar in range(2):
                            t = 2 * tp + tpar
                            stag = "s2a" if T % 2 == 0 else "s2b"
                            S_t = spool.tile([P, 2, 512], FP32, tag=stag,
                                             name=f"st{T}")
                            for hi in range(2):
                                nc.tensor.matmul(
                                    S_t[:, hi, :],
                                    k_sb[HD * hi:HD * hi + HD, pr,
                                         P * t:P * t + P],
                                    q_sb[HD * hi:HD * hi + HD, pr,
                                         512 * half:512 * half + 512],
                                    start=True, stop=True)
                            if sch:
                                nc.vector.tensor_scalar(
                                    E_i[:, tpar, :, :], S_t,
                                    A_SCH, B_SCH - SHIFT * A_SCH,
                                    OP.mult, OP.add)
                            elif USE_FP8:
                                nc.scalar.activation(E_sup[:, tpar, :, :],
                                                     S_t, AF.Exp,
                                                     bias=shift_sb,
                                                     scale=1.0)
                            else:
                                nc.scalar.activation(E_sup[:, tpar, :, :],
                                                     S_t, AF.Exp)
                            T += 1
                            # AV pairs flush with ~2-tile lag; drain hard
                            # near the stream end
                            lag = 2 if T < 57 else 0
                            while pend and pend[0][0] <= T - lag:
                                pend.pop(0)[1]()
                            for ci in range(fired, T):
                                for fn, args in drip.pop(ci, ()):
                                    fn(*args)
                            fired = T
                        pend.append((
                            T,
                            (lambda a=pr, b=half, c=tp, e=E_sup,
                             f=is_f8: emit_av_pair(a, b, c, e, f))))
                    while pend:
                        pend.pop(0)[1]()
                    emit_epilogue(pr, half, last=(half == 1 and pr == 3))
            assert not drip, drip

            # ---------------- tail: half-1 proj finals ----------------
            with nc.named_scope("proj"):
                for g in range(2):
                    ps2 = spool.tile([P, 2, 512], FP32,
                                     tag="s2a" if g == 0 else "s2b",
                                     name=f"pjt{g}")
                    for rr in range(2):
                        proj_mms(ps2[:, rr, :], 2 * g + rr, 1, 1)
                    o2 = outp.tile([P, 2, 512], FP32, tag="outsb2",
                                   name=f"osb2_{g}")
                    nc.vector.tensor_tensor(
                        o2, ps2, xpb_sb[:, 2 * g:2 * g + 2, 512:1024],
                        OP.add)
                    eng = nc.sync if g == 0 else nc.gpsimd
                    eng.dma_start(out_r[:, 2 * g:2 * g + 2, 512:1024], o2)

_CACHE: dict = {}


def _build():
    if "nc" in _CACHE:
        return _CACHE["nc"]
    nc = bacc.Bacc("TRN2", target_bir_lowering=False, debug=False,
                   num_devices=NCORES)
    wdt = F8 if USE_FP8 else BF16
    io = {
        "x": nc.dram_tensor("x", [C, NT], FP32, kind="ExternalInput").ap(),
        "wq": nc.dram_tensor("wq", [C, C], BF16, kind="ExternalInput").ap(),
        "wk": nc.dram_tensor("wk", [C, C], BF16, kind="ExternalInput").ap(),
        "wv": nc.dram_tensor("wv", [C, C], wdt, kind="ExternalInput").ap(),
        "pw": nc.dram_tensor("pw", [C, C], wdt, kind="ExternalInput").ap(),
        "smalls": nc.dram_tensor("smalls", [P, 28], FP32,
                                 kind="ExternalInput").ap(),
        "imat": nc.dram_tensor("imat", [NH, P], FP32,
                               kind="ExternalInput").ap(),
        "out": nc.dram_tensor("out", [C, NT], FP32, kind="ExternalOutput").ap(),
    }
    with tile.TileContext(nc) as tc:
        _emit(tc, io)
    nc.compile()
    _CACHE["nc"] = nc
    return nc


def _host_prep(inputs):
    x = np.ascontiguousarray(np.asarray(inputs["x"], dtype=np.float32))
    qkv_w = np.asarray(inputs["qkv_w"], dtype=np.float32)
    qkv_b = np.asarray(inputs["qkv_b"], dtype=np.float32)
    proj_w = np.asarray(inputs["proj_w"], dtype=np.float32)
    proj_b = np.asarray(inputs["proj_b"], dtype=np.float32)
    gn_scale = np.asarray(inputs["gn_scale"], dtype=np.float32)
    gn_bias = np.asarray(inputs["gn_bias"], dtype=np.float32)

    s = np.float32(1.0 / np.sqrt(HD))
    bf = ml_dtypes.bfloat16
    f8 = ml_dtypes.float8_e4m3 if USE_FP8 else bf

    def col(a):  # [(r p)] -> [p, r]
        return np.ascontiguousarray(a.reshape(CT, P).T)

    smalls = np.concatenate([
        col(gn_scale), col(gn_bias), col(qkv_b[0:C] * s), col(qkv_b[C:2 * C]),
        col((proj_b + proj_w @ qkv_b[2 * C:3 * C]).astype(np.float32)),
        # amat: [128, 8], 1/16 where channel p belongs to group j of its tile
        np.kron(np.eye(NH, dtype=np.float32),
                np.ones((GSZ, 1), np.float32)) / GSZ,
    ], axis=1).astype(np.float32)

    shared = {
        "wq": np.ascontiguousarray((qkv_w[0:C] * s).T).astype(bf),
        "wk": np.ascontiguousarray(qkv_w[C:2 * C].T).astype(bf),
        "wv": np.ascontiguousarray(qkv_w[2 * C:3 * C].T).astype(f8),
        "pw": np.ascontiguousarray(proj_w.T).astype(f8),
        "smalls": smalls,
        # imat: [8, 128], 1.0 where channel p belongs to group j of its tile
        "imat": np.ascontiguousarray(np.kron(np.eye(NH, dtype=np.float32),
                                             np.ones((1, GSZ), np.float32))),
    }
    B = x.shape[0]
    in_maps = []
    for b in range(B):
        m = dict(shared)
        m["x"] = np.ascontiguousarray(x[b].reshape(C, NT))
        in_maps.append(m)
    return in_maps


def run(inputs, trace=False):
    nc = _build()
    in_maps = _host_prep(inputs)
    res = run_bass_kernel_spmd(nc, in_maps, list(range(NCORES)), trace=trace)
    out = np.stack([res.results[i]["out"] for i in range(NCORES)], axis=0)
    return out.reshape(len(in_maps), C, 32, 32), res


def kernel(**inputs) -> np.ndarray:
    out, _ = run(inputs, trace=False)
    return out.astype(np.float32)


# revision 16
# speedup vs baseline: 1.4803x; 1.4803x over previous
"""Trainium2 Bass kernel for nn_AttentionBlock (GroupNorm -> MHA -> proj + residual).

Contract: kernel(**inputs) takes the FULL unsharded inputs (as produced by
setup_inputs) and returns the FULL output [8, 512, 32, 32] float32.

Sharding: pure data-parallel over batch B=8 across the 8 NeuronCores; each core
processes one batch element end-to-end (no collectives needed).

Per-core layout / algorithm (B=1, C=512, N=H*W=1024, heads=8, head_dim=64):
  - Head: x arrives as 8 half-tile DMAs spread over 4 engine queues; all small
    tensors ride ONE packed [128,28] DMA; dummy matmuls on a memset tile keep
    the PE HAM-warm (2.4 GHz) through the DMA window so the real stream never
    pays the 1.2 GHz cold clock.
  - GroupNorm: per-channel bn_stats chase the x half-DMAs; group-combine +
    broadcast via tiny PE matmuls split r0..2 / r3 so h0..2 finish before x3
    even lands; normalize splits across ScalarE/VectorE; h also cast to fp8
    on GpSimd for the v/proj path.
  - qkv 1x1-conv as matmuls with host-pre-transposed weights; q scale folded.
    q/k stay bf16 (fp8 there fails the error budget); wv/pw are fp8 e4m3 and
    their matmuls run DoubleRow over kc-tile pairs (2x fewer PE passes).
  - Attention (half-major order: token-half outer, head-pair inner, so the
    first half's proj + output DMA drain mid-stream). S^T per head via
    lhsT=k row-tiled pairs (hi=0/1 concurrent in the 128x128 array).
    exp(S-5) on ScalarE straight to fp8 e4m3 (softmax is shift-invariant;
    shift keeps e^ under the 240 fp8 max). A@V runs DoubleRow over m-tile
    pairs with lhsT = [ones|v] fp8, so denominators come out broadcast at
    PSUM partitions 0:64 for the custom-DVE fast reciprocal. A subset of
    pairs offloads exp to VectorE via the Schraudolph int16 trick (bf16 E,
    plain matmuls) to balance ScalarE vs PE.
  - proj matmuls DoubleRow over kc pairs from fp8 O; x+pb precomputed on
    GpSimd; residual add + output DMA per token-half.
"""

import numpy as np
import ml_dtypes

import concourse.bass as bass
import concourse.tile as tile
from concourse import bacc, mybir
from concourse.bass_utils import run_bass_kernel_spmd

FP32 = mybir.dt.float32
BF16 = mybir.dt.bfloat16
F8 = mybir.dt.float8e4
I16 = mybir.dt.int16
AF = mybir.ActivationFunctionType
OP = mybir.AluOpType
DR = mybir.MatmulPerfMode.DoubleRow

P = 128      # SBUF partitions
C = 512      # channels
NT = 1024    # spatial tokens (32*32)
CT = C // P  # channel tiles = 4
MT = NT // P # m (key) tiles = 8
NH = 8       # heads
HD = 64      # head dim
NCORES = 8
GSZ = 16     # channels per group (512/32)

USE_FP8 = True
SHIFT = 5.0          # exp(S - SHIFT): keeps e^ <= ~240 (fp8 max) w/ margin
WARM_MMS = 26        # dummy matmuls to hold the PE HAM-warm through DMA-in
# global pair indices (0..31) whose exp runs on VectorE (Schraudolph int16).
SCH_PAIRS = frozenset()
A_SCH = 128.0 / float(np.log(2.0))
B_SCH = 127.0 * 128.0 - 5.5


def _emit(tc: "tile.TileContext", io: dict):
    nc = tc.nc
    x, wq, wk, wv, pw = io["x"], io["wq"], io["wk"], io["wv"], io["pw"]
    smalls, imat = io["smalls"], io["imat"]
    out = io["out"]
    wdt = F8 if USE_FP8 else BF16

    import contextlib
    ctx = contextlib.ExitStack()
    with ctx:
        pers = ctx.enter_context(tc.tile_pool(name="pers", bufs=1))
        sm = ctx.enter_context(tc.tile_pool(name="small", bufs=1))

        # ---------------- input DMAs ----------------
        # x half-tiles spread over 4 queues so GN stats chase arrivals;
        # weights next on the queues that are otherwise idle; smalls packed.
        x_r = x.rearrange("(r p) n -> p r n", p=P)
        x_sb = pers.tile([P, CT, NT], FP32, tag="x")
        dummy = pers.tile([P, 512], BF16, tag="dummy")
        nc.gpsimd.memset(dummy, 0.5)  # first: unblocks the PE warm spam
        # x tiles 0-2 one per DMA queue; tile 3 (GN's critical tail) split
        # in halves across two queues so its stats start earliest
        nc.sync.dma_start(x_sb[:, 0, :], x_r[:, 0, :])
        nc.gpsimd.dma_start(x_sb[:, 1, :], x_r[:, 1, :])
        nc.scalar.dma_start(x_sb[:, 2, :], x_r[:, 2, :])
        nc.sync.dma_start(x_sb[:, 3, 0:512], x_r[:, 3, 0:512])
        nc.gpsimd.dma_start(x_sb[:, 3, 512:1024], x_r[:, 3, 512:1024])
        sm_sb = pers.tile([P, 28], FP32, tag="smalls")
        nc.scalar.dma_start(sm_sb, smalls)
        imat_sb = pers.tile([NH, P], FP32, tag="imat")
        nc.scalar.dma_start(imat_sb, imat)
        gg_sb, gb_sb = sm_sb[:, 0:4], sm_sb[:, 4:8]
        bq_sb, bk_sb = sm_sb[:, 8:12], sm_sb[:, 12:16]
        pb_sb, amat_sb = sm_sb[:, 16:20], sm_sb[:, 20:28]
        wq_sb = pers.tile([P, CT, C], BF16, tag="wq")
        nc.scalar.dma_start(wq_sb, wq.rearrange("(k p) o -> p k o", p=P))
        wk_sb = pers.tile([P, CT, C], BF16, tag="wk")
        nc.sync.dma_start(wk_sb, wk.rearrange("(k p) o -> p k o", p=P))
        wv_sb = pers.tile([P, CT, C], wdt, tag="wv")
        nc.gpsimd.dma_start(wv_sb, wv.rearrange("(k p) o -> p k o", p=P))
        pw_sb = pers.tile([P, CT, C], wdt, tag="pw")
        nc.sync.dma_start(pw_sb, pw.rearrange("(k p) o -> p k o", p=P))
        # preload the exp activation table while DMAs are in flight
        warm_sb = pers.tile([1, 1], FP32, tag="actwarm")
        nc.vector.memset(warm_sb, 0.0)
        nc.scalar.activation(warm_sb, warm_sb, AF.Exp)
        # per-partition -SHIFT bias column for the shifted exp
        shift_sb = pers.tile([P, 1], FP32, tag="shift")
        nc.vector.memset(shift_sb, -SHIFT)

        # v^T with interleaved ones columns: per head 128 cols = [ones(64) | v(64)]
        vT_sb = pers.tile([P, MT, NH * 128], wdt, tag="vT")
        nc.gpsimd.memset(
            vT_sb.rearrange("p t (h c) -> p t h c", c=128)[:, :, :, 0:HD], 1.0)

        h_sb = pers.tile([P, CT, NT], BF16, tag="h")
        if USE_FP8:
            h8_sb = pers.tile([P, CT, NT], wdt, tag="h8", name="h8")
        else:
            h8_sb = h_sb
        q_sb = pers.tile([P, CT, NT], BF16, tag="q")
        k_sb = pers.tile([P, CT, NT], BF16, tag="k")
        O_sb = pers.tile([P, CT, NT], wdt, tag="O")
        xpb_sb = pers.tile([P, CT, NT], FP32, tag="xpb")

        # ---------------- PE warm spam ----------------
        # The HAM clock gate defaults to 1.2 GHz and only opens to 2.4 GHz
        # after ~3.4us of sustained PE activity. Burn idle DMA-wait time on
        # dummy matmuls so the real stream starts (and stays) warm.
        with tc.tile_pool(name="warmps", bufs=1, space="PSUM") as wps:
            wt = wps.tile([P, 512], FP32, tag="warm")
            for _ in range(WARM_MMS):
                nc.tensor.matmul(wt, dummy[:, 0:128], dummy,
                                 start=True, stop=True)

        # ---------------- GroupNorm (per-tile pipelined) ----------------
        # groups are 16 channels wide so every group lives inside one
        # 128-channel tile; tiles are fully independent. r0..2 finish while
        # x3 is still in flight; r3 runs the short critical chain alone.
        with nc.named_scope("gn"), \
             tc.tile_pool(name="gnps", bufs=1, space="PSUM") as gnps:
            stats = sm.tile([P, CT, 2, 6], FP32, tag="bnst")
            mv = sm.tile([P, CT, 2], FP32, tag="gnmv")
            st2 = sm.tile([P, CT, 2], FP32, tag="gnst2")
            for r in range(CT):
                nc.vector.bn_stats(stats[:, r, 0, :], x_sb[:, r, 0:512])
                nc.vector.bn_stats(stats[:, r, 1, :], x_sb[:, r, 512:1024])
                nc.vector.bn_aggr(mv[:, r, :], stats[:, r])
                nc.vector.tensor_copy(st2[:, r, 0:1], mv[:, r, 0:1])
                nc.vector.tensor_tensor(st2[:, r, 1:2], mv[:, r, 0:1],
                                        mv[:, r, 0:1], OP.mult)
                nc.vector.tensor_tensor(st2[:, r, 1:2], st2[:, r, 1:2],
                                        mv[:, r, 1:2], OP.add)
            G_ps = gnps.tile([NH, CT, 2], FP32, tag="gps")
            MR_ps = gnps.tile([P, CT, 2], FP32, tag="mrps")
            st_all = sm.tile([NH, CT, 2], FP32, tag="gnsta")
            var_all = sm.tile([NH, CT], FP32, tag="gnvar")
            y_t = sm.tile([NH, CT, 2], FP32, tag="gnyt")
            mr = sm.tile([P, CT, 2], FP32, tag="gnmr")
            ab = sm.tile([P, CT, 2], FP32, tag="gnab")

            def gn_finish(lo, hi):
                sl = slice(lo, hi)
                # per-group (mean, E[x^2]) for tiles [lo,hi) in one matmul
                nc.tensor.matmul(G_ps[:, sl, :], amat_sb, st2[:, sl, :],
                                 start=True, stop=True)
                nc.vector.tensor_copy(st_all[:, sl, :], G_ps[:, sl, :])
                nc.vector.tensor_tensor(var_all[:, sl, None],
                                        st_all[:, sl, 0:1],
                                        st_all[:, sl, 0:1], OP.mult)
                nc.vector.tensor_tensor(var_all[:, sl, None],
                                        st_all[:, sl, 1:2],
                                        var_all[:, sl, None], OP.subtract)
                nc.vector.tensor_scalar(var_all[:, sl], var_all[:, sl],
                                        1e-5, None, OP.add)
                y = y_t[:, sl, 0:1]
                t = y_t[:, sl, 1:2]
                va = var_all[:, sl, None]
                nc.vector.reciprocal_approx_fast(y, va)
                for it in range(2):
                    nc.vector.tensor_tensor(t, y, y, OP.mult)
                    nc.vector.tensor_tensor(t, t, va, OP.mult)
                    nc.vector.tensor_scalar(t, t, -0.5, 1.5, OP.mult, OP.add)
                    if it < 1:
                        nc.vector.tensor_tensor(y, y, t, OP.mult)
                    else:
                        nc.vector.tensor_tensor(st_all[:, sl, 1:2], y, t,
                                                OP.mult)
                # broadcast (mean, rstd) back to channels
                nc.tensor.matmul(MR_ps[:, sl, :], imat_sb, st_all[:, sl, :],
                                 start=True, stop=True)
                nc.vector.tensor_copy(mr[:, sl, :], MR_ps[:, sl, :])
                nc.vector.tensor_tensor(ab[:, sl, 0:1], mr[:, sl, 1:2],
                                        gg_sb[:, sl, None], OP.mult)
                nc.vector.tensor_tensor(ab[:, sl, 1:2], mr[:, sl, 0:1],
                                        ab[:, sl, 0:1], OP.mult)
                nc.vector.tensor_tensor(ab[:, sl, 1:2], gb_sb[:, sl, None],
                                        ab[:, sl, 1:2], OP.subtract)
                for r in range(lo, hi):
                    if r < CT - 1:
                        nc.scalar.activation(h_sb[:, r, :], x_sb[:, r, :],
                                             AF.Identity, bias=ab[:, r, 1:2],
                                             scale=ab[:, r, 0:1])
                    else:
                        # last tile: split halves across ScalarE/VectorE to
                        # shorten the critical path into the first qkv matmul
                        nc.scalar.activation(h_sb[:, r, 0:512],
                                             x_sb[:, r, 0:512],
                                             AF.Identity, bias=ab[:, r, 1:2],
                                             scale=ab[:, r, 0:1])
                        nc.vector.tensor_scalar(h_sb[:, r, 512:1024],
                                                x_sb[:, r, 512:1024],
                                                ab[:, r, 0:1], ab[:, r, 1:2],
                                                OP.mult, OP.add)
                    if USE_FP8:
                        nc.vector.tensor_copy(h8_sb[:, r, :], h_sb[:, r, :])

            gn_finish(0, CT - 1)
            gn_finish(CT - 1, CT)

        # ------------- qkv + attention (interleaved on PE) -------------
        # PSUM (8 banks): S chunks [128,2,512] x2 tags (4) + O pair-half
        # [128,2,512] (2) + background qkv/proj accumulators [128,512] x2 (2).
        with nc.named_scope("qkv_attn"), \
             tc.tile_pool(name="bgps", bufs=2, space="PSUM") as bgps, \
             tc.tile_pool(name="spool", bufs=1, space="PSUM") as spool, \
             tc.tile_pool(name="opool", bufs=1, space="PSUM") as opool, \
             tc.tile_pool(name="epool", bufs=3) as epool, \
             tc.tile_pool(name="rpool", bufs=2) as rpool, \
             tc.tile_pool(name="outp", bufs=4) as outp:

            def qk_task(dst, w_sb, b_sb, r, half, on_act=False):
                ps = bgps.tile([P, 512], FP32, tag="bgps",
                               name=f"qk_{r}_{half}_{w_sb.name}")
                for kc in range(CT):
                    nc.tensor.matmul(
                        ps, w_sb[:, kc, P * r:P * r + P],
                        h_sb[:, kc, 512 * half:512 * half + 512],
                        start=(kc == 0), stop=(kc == CT - 1))
                dsl = dst[:, r, 512 * half:512 * half + 512]
                if on_act:
                    nc.scalar.activation(dsl, ps, AF.Identity,
                                         bias=b_sb[:, r:r + 1], scale=1.0)
                else:
                    nc.vector.tensor_scalar(dsl, ps, b_sb[:, r:r + 1],
                                            None, OP.add)

            def vt_task(t):
                ps = bgps.tile([P, 512], FP32, tag="bgps", name=f"vt{t}")
                if USE_FP8:
                    for s in range(CT // 2):
                        nc.tensor.matmul(
                            ps, h8_sb[:, 2 * s:2 * s + 2, P * t:P * t + P],
                            wv_sb[:, 2 * s:2 * s + 2, :],
                            start=(s == 0), stop=(s == CT // 2 - 1),
                            perf_mode=DR)
                else:
                    for kc in range(CT):
                        nc.tensor.matmul(ps, h8_sb[:, kc, P * t:P * t + P],
                                         wv_sb[:, kc, :],
                                         start=(kc == 0), stop=(kc == CT - 1))
                nc.vector.tensor_copy(
                    vT_sb[:, t, :].rearrange("p (h c) -> p h c", c=128)[:, :, HD:128],
                    ps.rearrange("p (h c) -> p h c", c=HD))

            def xpb_task(rr, half):
                hs = 512 * half
                nc.vector.tensor_scalar(xpb_sb[:, rr, hs:hs + 512],
                                        x_sb[:, rr, hs:hs + 512],
                                        pb_sb[:, rr:rr + 1], None, OP.add)

            out_r = out.rearrange("(r p) n -> p r n", p=P)

            def proj_mms(ps, r, half, lo_pair):
                hs = 512 * half
                s = lo_pair
                if USE_FP8:
                    nc.tensor.matmul(
                        ps, pw_sb[:, 2 * s:2 * s + 2, P * r:P * r + P],
                        O_sb[:, 2 * s:2 * s + 2, hs:hs + 512],
                        start=True, stop=True, perf_mode=DR)
                else:
                    for kc in (2 * s, 2 * s + 1):
                        nc.tensor.matmul(
                            ps, pw_sb[:, kc, P * r:P * r + P],
                            O_sb[:, kc, hs:hs + 512],
                            start=(kc == 2 * s), stop=(kc == 2 * s + 1))

            def proj_part(r, half):
                # kc pair (0,1): heads 0..3 of this token half + x + pb,
                # accumulated in place into xpb
                hs = 512 * half
                ps = bgps.tile([P, 512], FP32, tag="bgps",
                               name=f"pp{r}_{half}")
                proj_mms(ps, r, half, 0)
                nc.vector.tensor_tensor(xpb_sb[:, r, hs:hs + 512], ps,
                                        xpb_sb[:, r, hs:hs + 512], OP.add)

            def proj_fin(r, half, eng_i=0):
                hs = 512 * half
                ps = bgps.tile([P, 512], FP32, tag="bgps",
                               name=f"pf{r}_{half}")
                proj_mms(ps, r, half, 1)
                o_sb = outp.tile([P, 512], FP32, tag="outsb",
                                 name=f"osb{r}_{half}")
                nc.vector.tensor_tensor(o_sb, ps,
                                        xpb_sb[:, r, hs:hs + 512], OP.add)
                eng = (nc.sync, nc.gpsimd)[eng_i % 2]
                eng.dma_start(out_r[:, r, hs:hs + 512], o_sb)

            # upfront: only what attention tile 0 needs (q0/k0 first halves)
            qk_task(q_sb, wq_sb, bq_sb, 0, 0, on_act=True)
            qk_task(k_sb, wk_sb, bk_sb, 0, 0, on_act=True)

            # drip key k fires right after stream tile T=k+1; keys 7,15,...
            # land just before a head-pair boundary where the PE otherwise
            # stalls on the last exp, so they hold PE-heavy tasks.
            drip = {
                0: [(qk_task, (q_sb, wq_sb, bq_sb, 1, 0))],
                1: [(qk_task, (k_sb, wk_sb, bk_sb, 1, 0))],
                2: [(vt_task, (0,)), (vt_task, (1,))],
                3: [(vt_task, (2,)), (vt_task, (3,))],
                4: [(vt_task, (4,))], 5: [(vt_task, (5,))],
                6: [(vt_task, (6,))], 7: [(vt_task, (7,))],
                8: [(xpb_task, (0, 0)), (xpb_task, (1, 0))],
                9: [(xpb_task, (2, 0)), (xpb_task, (3, 0))],
                10: [(qk_task, (q_sb, wq_sb, bq_sb, 2, 0))],
                11: [(qk_task, (k_sb, wk_sb, bk_sb, 2, 0))],
                15: [(qk_task, (q_sb, wq_sb, bq_sb, 3, 0))],
                16: [(qk_task, (k_sb, wk_sb, bk_sb, 3, 0))],
                20: [(proj_part, (0, 0))], 21: [(proj_part, (1, 0))],
                22: [(proj_part, (2, 0))], 23: [(proj_part, (3, 0))],
                26: [(qk_task, (q_sb, wq_sb, bq_sb, 0, 1))],
                27: [(qk_task, (k_sb, wk_sb, bk_sb, 0, 1))],
                32: [(proj_fin, (0, 0, 0))], 33: [(proj_fin, (1, 0, 1))],
                34: [(qk_task, (q_sb, wq_sb, bq_sb, 1, 1))],
                35: [(qk_task, (k_sb, wk_sb, bk_sb, 1, 1))],
                36: [(proj_fin, (2, 0, 0))], 37: [(proj_fin, (3, 0, 1))],
                39: [(qk_task, (q_sb, wq_sb, bq_sb, 2, 1))],
                40: [(qk_task, (k_sb, wk_sb, bk_sb, 2, 1))],
                42: [(xpb_task, (0, 1)), (xpb_task, (1, 1))],
                43: [(xpb_task, (2, 1)), (xpb_task, (3, 1))],
                46: [(qk_task, (q_sb, wq_sb, bq_sb, 3, 1))],
                47: [(qk_task, (k_sb, wk_sb, bk_sb, 3, 1))],
                52: [(proj_part, (0, 1))], 53: [(proj_part, (1, 1))],
                54: [(proj_part, (2, 1))], 55: [(proj_part, (3, 1))],
            }

            # ---- the unit stream: half-major, AV in DoubleRow m-tile pairs
            O_cur = [None]

            def emit_av_pair(pr, half, tp, E_sup, is_f8):
                for hi in range(2):
                    h8c = 128 * (2 * pr + hi)
                    if is_f8:
                        nc.tensor.matmul(
                            O_cur[0][:, hi, :],
                            vT_sb[:, 2 * tp:2 * tp + 2, h8c:h8c + 128],
                            E_sup[:, :, hi, :],
                            start=(tp == 0), stop=(tp == 3),
                            perf_mode=DR)
                    else:
                        for tpar in range(2):
                            nc.tensor.matmul(
                                O_cur[0][:, hi, :],
                                vT_sb[:, 2 * tp + tpar, h8c:h8c + 128],
                                E_sup[:, tpar, hi, :],
                                start=(tp == 0 and tpar == 0),
                                stop=(tp == 3 and tpar == 1))

            def emit_epilogue(pr, half, last=False):
                hs = 512 * half
                O_half = O_cur[0]
                Rh = rpool.tile([HD, 2, 512], FP32, tag="rh",
                                name=f"rh{pr}_{half}")
                if last:
                    # split per-hi so the final proj matmuls start sooner
                    for hi in range(2):
                        nc.vector.reciprocal_approx_fast(
                            Rh[:, hi, :], O_half[0:HD, hi, :])
                        nc.vector.tensor_tensor(
                            O_sb[HD * hi:HD * hi + HD, pr, hs:hs + 512],
                            O_half[HD:128, hi, :], Rh[:, hi, :], OP.mult)
                else:
                    nc.vector.reciprocal_approx_fast(Rh, O_half[0:HD, :, :])
                    for hi in range(2):
                        nc.vector.tensor_tensor(
                            O_sb[HD * hi:HD * hi + HD, pr, hs:hs + 512],
                            O_half[HD:128, hi, :], Rh[:, hi, :], OP.mult)

            pend = []  # deferred AV pair flushes: (tile_due, closure)
            T = 0
            fired = 0
            for half in range(2):
                for pr in range(4):
                    O_cur[0] = opool.tile([P, 2, 512], FP32, tag="oh",
                                          name=f"oh{pr}_{half}")
                    for tp in range(4):
                        gpi = (4 * half + pr) * 4 + tp
                        sch = gpi in SCH_PAIRS
                        is_f8 = USE_FP8 and not sch
                        if is_f8:
                            E_sup = epool.tile([P, 2, 2, 512], F8, tag="e8",
                                               name=f"e{gpi}")
                        else:
                            E_i = epool.tile([P, 2, 2, 512], I16, tag="ebf",
                                             name=f"e{gpi}")
                            E_sup = E_i.bitcast(BF16)
                        for tpar in range(2):
                            t = 2 * tp + tpar
                            stag = "s2a" if T % 2 == 0 else "s2b"
                            S_t = spool.tile([P, 2, 512], FP32, tag=stag,
                                             name=f"st{T}")
                            for hi in range(2):
                                nc.tensor.matmul(
                                    S_t[:, hi, :],
                                    k_sb[HD * hi:HD * hi + HD, pr,
                                         P * t:P * t + P],
                                    q_sb[HD * hi:HD * hi + HD, pr,
                                         512 * half:512 * half + 512],
                                    start=True, stop=True)
                            if sch:
                                nc.vector.tensor_scalar(
                                    E_i[:, tpar, :, :], S_t,
                                    A_SCH, B_SCH - SHIFT * A_SCH,
                                    OP.mult, OP.add)
                            elif USE_FP8:
                                nc.scalar.activation(E_sup[:, tpar, :, :],
                                                     S_t, AF.Exp,
                                                     bias=shift_sb,
                                                     scale=1.0)
                            else:
                                nc.scalar.activation(E_sup[:, tpar, :, :],
                                                     S_t, AF.Exp)
                            T += 1
                            # AV pairs flush with ~2-tile lag; drain hard
                            # near the stream end
                            lag = 3 if T < 16 else (2 if T < 57 else 0)
                            while pend and pend[0][0] <= T - lag:
                                pend.pop(0)[1]()
                            for ci in range(fired, T):
                                for fn, args in drip.pop(ci, ()):
                                    fn(*args)
                            fired = T
                        pend.append((
                            T,
                            (lambda a=pr, b=half, c=tp, e=E_sup,
                             f=is_f8: emit_av_pair(a, b, c, e, f))))
                    while pend:
                        pend.pop(0)[1]()
                    emit_epilogue(pr, half, last=(half == 1 and pr == 3))
            assert not drip, drip

            # ---------------- tail: half-1 proj finals ----------------
            with nc.named_scope("proj"):
                for g in range(2):
                    ps2 = spool.tile([P, 2, 512], FP32,
                                     tag="s2a" if g == 0 else "s2b",
                                     name=f"pjt{g}")
                    for rr in range(2):
                        proj_mms(ps2[:, rr, :], 2 * g + rr, 1, 1)
                    o2 = outp.tile([P, 2, 512], FP32, tag="outsb2",
                                   name=f"osb2_{g}")
                    nc.vector.tensor_tensor(
                        o2, ps2, xpb_sb[:, 2 * g:2 * g + 2, 512:1024],
                        OP.add)
                    eng = nc.sync if g == 0 else nc.gpsimd
                    eng.dma_start(out_r[:, 2 * g:2 * g + 2, 512:1024], o2)

_CACHE: dict = {}


def _build():
    if "nc" in _CACHE:
        return _CACHE["nc"]
    nc = bacc.Bacc("TRN2", target_bir_lowering=False, debug=False,
                   num_devices=NCORES)
    wdt = F8 if USE_FP8 else BF16
    io = {
        "x": nc.dram_tensor("x", [C, NT], FP32, kind="ExternalInput").ap(),
        "wq": nc.dram_tensor("wq", [C, C], BF16, kind="ExternalInput").ap(),
        "wk": nc.dram_tensor("wk", [C, C], BF16, kind="ExternalInput").ap(),
        "wv": nc.dram_tensor("wv", [C, C], wdt, kind="ExternalInput").ap(),
        "pw": nc.dram_tensor("pw", [C, C], wdt, kind="ExternalInput").ap(),
        "smalls": nc.dram_tensor("smalls", [P, 28], FP32,
                                 kind="ExternalInput").ap(),
        "imat": nc.dram_tensor("imat", [NH, P], FP32,
                               kind="ExternalInput").ap(),
        "out": nc.dram_tensor("out", [C, NT], FP32, kind="ExternalOutput").ap(),
    }
    with tile.TileContext(nc) as tc:
        _emit(tc, io)
    nc.compile()
    _CACHE["nc"] = nc
    return nc


def _host_prep(inputs):
    x = np.ascontiguousarray(np.asarray(inputs["x"], dtype=np.float32))
    qkv_w = np.asarray(inputs["qkv_w"], dtype=np.float32)
    qkv_b = np.asarray(inputs["qkv_b"], dtype=np.float32)
    proj_w = np.asarray(inputs["proj_w"], dtype=np.float32)
    proj_b = np.asarray(inputs["proj_b"], dtype=np.float32)
    gn_scale = np.asarray(inputs["gn_scale"], dtype=np.float32)
    gn_bias = np.asarray(inputs["gn_bias"], dtype=np.float32)

    s = np.float32(1.0 / np.sqrt(HD))
    bf = ml_dtypes.bfloat16
    f8 = ml_dtypes.float8_e4m3 if USE_FP8 else bf

    def col(a):  # [(r p)] -> [p, r]
        return np.ascontiguousarray(a.reshape(CT, P).T)

    smalls = np.concatenate([
        col(gn_scale), col(gn_bias), col(qkv_b[0:C] * s), col(qkv_b[C:2 * C]),
        col((proj_b + proj_w @ qkv_b[2 * C:3 * C]).astype(np.float32)),
        # amat: [128, 8], 1/16 where channel p belongs to group j of its tile
        np.kron(np.eye(NH, dtype=np.float32),
                np.ones((GSZ, 1), np.float32)) / GSZ,
    ], axis=1).astype(np.float32)

    shared = {
        "wq": np.ascontiguousarray((qkv_w[0:C] * s).T).astype(bf),
        "wk": np.ascontiguousarray(qkv_w[C:2 * C].T).astype(bf),
        "wv": np.ascontiguousarray(qkv_w[2 * C:3 * C].T).astype(f8),
        "pw": np.ascontiguousarray(proj_w.T).astype(f8),
        "smalls": smalls,
        # imat: [8, 128], 1.0 where channel p belongs to group j of its tile
        "imat": np.ascontiguousarray(np.kron(np.eye(NH, dtype=np.float32),
                                             np.ones((1, GSZ), np.float32))),
    }
    B = x.shape[0]
    in_maps = []
    for b in range(B):
        m = dict(shared)
        m["x"] = np.ascontiguousarray(x[b].reshape(C, NT))
        in_maps.append(m)
    return in_maps


def run(inputs, trace=False):
    nc = _build()
    in_maps = _host_prep(inputs)
    res = run_bass_kernel_spmd(nc, in_maps, list(range(NCORES)), trace=trace)
    out = np.stack([res.results[i]["out"] for i in range(NCORES)], axis=0)
    return out.reshape(len(in_maps), C, 32, 32), res


def kernel(**inputs) -> np.ndarray:
    out, _ = run(inputs, trace=False)
    return out.astype(np.float32)


# revision 25
# speedup vs baseline: 1.4961x; 1.0107x over previous
"""Trainium2 Bass kernel for nn_AttentionBlock (GroupNorm -> MHA -> proj + residual).

Contract: kernel(**inputs) takes the FULL unsharded inputs (as produced by
setup_inputs) and returns the FULL output [8, 512, 32, 32] float32.

Sharding: pure data-parallel over batch B=8 across the 8 NeuronCores; each core
processes one batch element end-to-end (no collectives needed).

Per-core layout / algorithm (B=1, C=512, N=H*W=1024, heads=8, head_dim=64):
  - Head: x arrives as 8 half-tile DMAs spread over 4 engine queues; all small
    tensors ride ONE packed [128,28] DMA; dummy matmuls on a memset tile keep
    the PE HAM-warm (2.4 GHz) through the DMA window so the real stream never
    pays the 1.2 GHz cold clock.
  - GroupNorm: per-channel bn_stats chase the x half-DMAs; group-combine +
    broadcast via tiny PE matmuls split r0..2 / r3 so h0..2 finish before x3
    even lands; normalize splits across ScalarE/VectorE; h also cast to fp8
    on GpSimd for the v/proj path.
  - qkv 1x1-conv as matmuls with host-pre-transposed weights; q scale folded.
    q/k stay bf16 (fp8 there fails the error budget); wv/pw are fp8 e4m3 and
    their matmuls run DoubleRow over kc-tile pairs (2x fewer PE passes).
  - Attention (half-major order: token-half outer, head-pair inner, so the
    first half's proj + output DMA drain mid-stream). S^T per head via
    lhsT=k row-tiled pairs (hi=0/1 concurrent in the 128x128 array).
    exp(S-5) on ScalarE straight to fp8 e4m3 (softmax is shift-invariant;
    shift keeps e^ under the 240 fp8 max). A@V runs DoubleRow over m-tile
    pairs with lhsT = [ones|v] fp8, so denominators come out broadcast at
    PSUM partitions 0:64 for the custom-DVE fast reciprocal. A subset of
    pairs offloads exp to VectorE via the Schraudolph int16 trick (bf16 E,
    plain matmuls) to balance ScalarE vs PE.
  - proj matmuls DoubleRow over kc pairs from fp8 O; x+pb precomputed on
    GpSimd; residual add + output DMA per token-half.
"""

import numpy as np
import ml_dtypes

import concourse.bass as bass
import concourse.tile as tile
from concourse import bacc, mybir
from concourse.bass_utils import run_bass_kernel_spmd

FP32 = mybir.dt.float32
BF16 = mybir.dt.bfloat16
F8 = mybir.dt.float8e4
I16 = mybir.dt.int16
AF = mybir.ActivationFunctionType
OP = mybir.AluOpType
DR = mybir.MatmulPerfMode.DoubleRow

P = 128      # SBUF partitions
C = 512      # channels
NT = 1024    # spatial tokens (32*32)
CT = C // P  # channel tiles = 4
MT = NT // P # m (key) tiles = 8
NH = 8       # heads
HD = 64      # head dim
NCORES = 8
GSZ = 16     # channels per group (512/32)

USE_FP8 = True
SHIFT = 5.0          # exp(S - SHIFT): keeps e^ <= ~240 (fp8 max) w/ margin
WARM_MMS = 26        # dummy matmuls to hold the PE HAM-warm through DMA-in
# global pair indices (0..31) whose exp runs on VectorE (Schraudolph int16).
SCH_PAIRS = frozenset()
A_SCH = 128.0 / float(np.log(2.0))
B_SCH = 127.0 * 128.0 - 5.5


def _emit(tc: "tile.TileContext", io: dict):
    nc = tc.nc
    x, wq, wk, wv, pw = io["x"], io["wq"], io["wk"], io["wv"], io["pw"]
    smalls, imat = io["smalls"], io["imat"]
    out = io["out"]
    wdt = F8 if USE_FP8 else BF16

    import contextlib
    ctx = contextlib.ExitStack()
    with ctx:
        pers = ctx.enter_context(tc.tile_pool(name="pers", bufs=1))
        sm = ctx.enter_context(tc.tile_pool(name="small", bufs=1))

        # ---------------- input DMAs ----------------
        # x half-tiles spread over 4 queues so GN stats chase arrivals;
        # weights next on the queues that are otherwise idle; smalls packed.
        x_r = x.rearrange("(r p) n -> p r n", p=P)
        x_sb = pers.tile([P, CT, NT], FP32, tag="x")
        dummy = pers.tile([P, 512], BF16, tag="dummy")
        nc.gpsimd.memset(dummy, 0.5)  # first: unblocks the PE warm spam
        # x tiles 0-2 one per DMA queue; tile 3 (GN's critical tail) split
        # in halves across two queues so its stats start earliest
        nc.sync.dma_start(x_sb[:, 0, :], x_r[:, 0, :])
        nc.gpsimd.dma_start(x_sb[:, 1, :], x_r[:, 1, :])
        nc.scalar.dma_start(x_sb[:, 2, :], x_r[:, 2, :])
        nc.sync.dma_start(x_sb[:, 3, 0:512], x_r[:, 3, 0:512])
        nc.gpsimd.dma_start(x_sb[:, 3, 512:1024], x_r[:, 3, 512:1024])
        sm_sb = pers.tile([P, 28], FP32, tag="smalls")
        nc.scalar.dma_start(sm_sb, smalls)
        imat_sb = pers.tile([NH, P], FP32, tag="imat")
        nc.scalar.dma_start(imat_sb, imat)
        gg_sb, gb_sb = sm_sb[:, 0:4], sm_sb[:, 4:8]
        bq_sb, bk_sb = sm_sb[:, 8:12], sm_sb[:, 12:16]
        pb_sb, amat_sb = sm_sb[:, 16:20], sm_sb[:, 20:28]
        wq_sb = pers.tile([P, CT, C], BF16, tag="wq")
        nc.scalar.dma_start(wq_sb, wq.rearrange("(k p) o -> p k o", p=P))
        wk_sb = pers.tile([P, CT, C], BF16, tag="wk")
        nc.sync.dma_start(wk_sb, wk.rearrange("(k p) o -> p k o", p=P))
        wv_sb = pers.tile([P, CT, C], BF16, tag="wv")
        nc.gpsimd.dma_start(wv_sb, wv.rearrange("(k p) o -> p k o", p=P))
        pw_sb = pers.tile([P, CT, C], wdt, tag="pw")
        nc.sync.dma_start(pw_sb, pw.rearrange("(k p) o -> p k o", p=P))
        # preload the exp activation table while DMAs are in flight
        warm_sb = pers.tile([1, 1], FP32, tag="actwarm")
        nc.vector.memset(warm_sb, 0.0)
        nc.scalar.activation(warm_sb, warm_sb, AF.Exp)
        # per-partition -SHIFT bias column for the shifted exp
        shift_sb = pers.tile([P, 1], FP32, tag="shift")
        nc.vector.memset(shift_sb, -SHIFT)

        # v^T with interleaved ones columns: per head 128 cols = [ones(64) | v(64)]
        vT_sb = pers.tile([P, MT, NH * 128], wdt, tag="vT")
        nc.gpsimd.memset(
            vT_sb.rearrange("p t (h c) -> p t h c", c=128)[:, :, :, 0:HD], 1.0)

        h_sb = pers.tile([P, CT, NT], BF16, tag="h")
        q_sb = pers.tile([P, CT, NT], BF16, tag="q")
        k_sb = pers.tile([P, CT, NT], BF16, tag="k")
        O_sb = pers.tile([P, CT, NT], wdt, tag="O")
        xpb_sb = pers.tile([P, CT, NT], FP32, tag="xpb")

        # ---------------- PE warm spam ----------------
        # The HAM clock gate defaults to 1.2 GHz and only opens to 2.4 GHz
        # after ~3.4us of sustained PE activity. Burn idle DMA-wait time on
        # dummy matmuls so the real stream starts (and stays) warm.
        with tc.tile_pool(name="warmps", bufs=1, space="PSUM") as wps:
            wt = wps.tile([P, 512], FP32, tag="warm")
            for _ in range(WARM_MMS):
                nc.tensor.matmul(wt, dummy[:, 0:128], dummy,
                                 start=True, stop=True)

        # ---------------- GroupNorm (per-tile pipelined) ----------------
        # groups are 16 channels wide so every group lives inside one
        # 128-channel tile; tiles are fully independent. r0..2 finish while
        # x3 is still in flight; r3 runs the short critical chain alone.
        with nc.named_scope("gn"), \
             tc.tile_pool(name="gnps", bufs=1, space="PSUM") as gnps:
            stats = sm.tile([P, CT, 2, 6], FP32, tag="bnst")
            mv = sm.tile([P, CT, 2], FP32, tag="gnmv")
            st2 = sm.tile([P, CT, 2], FP32, tag="gnst2")
            for r in range(CT):
                nc.vector.bn_stats(stats[:, r, 0, :], x_sb[:, r, 0:512])
                nc.vector.bn_stats(stats[:, r, 1, :], x_sb[:, r, 512:1024])
                nc.vector.bn_aggr(mv[:, r, :], stats[:, r])
                nc.vector.tensor_copy(st2[:, r, 0:1], mv[:, r, 0:1])
                nc.vector.tensor_tensor(st2[:, r, 1:2], mv[:, r, 0:1],
                                        mv[:, r, 0:1], OP.mult)
                nc.vector.tensor_tensor(st2[:, r, 1:2], st2[:, r, 1:2],
                                        mv[:, r, 1:2], OP.add)
            G_ps = gnps.tile([NH, CT, 2], FP32, tag="gps")
            MR_ps = gnps.tile([P, CT, 2], FP32, tag="mrps")
            st_all = sm.tile([NH, CT, 2], FP32, tag="gnsta")
            var_all = sm.tile([NH, CT], FP32, tag="gnvar")
            y_t = sm.tile([NH, CT, 2], FP32, tag="gnyt")
            mr = sm.tile([P, CT, 2], FP32, tag="gnmr")
            ab = sm.tile([P, CT, 2], FP32, tag="gnab")

            def gn_finish(lo, hi):
                sl = slice(lo, hi)
                # per-group (mean, E[x^2]) for tiles [lo,hi) in one matmul
                nc.tensor.matmul(G_ps[:, sl, :], amat_sb, st2[:, sl, :],
                                 start=True, stop=True)
                nc.vector.tensor_copy(st_all[:, sl, :], G_ps[:, sl, :])
                nc.vector.tensor_tensor(var_all[:, sl, None],
                                        st_all[:, sl, 0:1],
                                        st_all[:, sl, 0:1], OP.mult)
                nc.vector.tensor_tensor(var_all[:, sl, None],
                                        st_all[:, sl, 1:2],
                                        var_all[:, sl, None], OP.subtract)
                nc.vector.tensor_scalar(var_all[:, sl], var_all[:, sl],
                                        1e-5, None, OP.add)
                y = y_t[:, sl, 0:1]
                t = y_t[:, sl, 1:2]
                va = var_all[:, sl, None]
                nc.vector.reciprocal_approx_fast(y, va)
                for it in range(2):
                    nc.vector.tensor_tensor(t, y, y, OP.mult)
                    nc.vector.tensor_tensor(t, t, va, OP.mult)
                    nc.vector.tensor_scalar(t, t, -0.5, 1.5, OP.mult, OP.add)
                    if it < 1:
                        nc.vector.tensor_tensor(y, y, t, OP.mult)
                    else:
                        nc.vector.tensor_tensor(st_all[:, sl, 1:2], y, t,
                                                OP.mult)
                # broadcast (mean, rstd) back to channels
                nc.tensor.matmul(MR_ps[:, sl, :], imat_sb, st_all[:, sl, :],
                                 start=True, stop=True)
                nc.vector.tensor_copy(mr[:, sl, :], MR_ps[:, sl, :])
                nc.vector.tensor_tensor(ab[:, sl, 0:1], mr[:, sl, 1:2],
                                        gg_sb[:, sl, None], OP.mult)
                nc.vector.tensor_tensor(ab[:, sl, 1:2], mr[:, sl, 0:1],
                                        ab[:, sl, 0:1], OP.mult)
                nc.vector.tensor_tensor(ab[:, sl, 1:2], gb_sb[:, sl, None],
                                        ab[:, sl, 1:2], OP.subtract)
                for r in range(lo, hi):
                    if r < CT - 1:
                        nc.scalar.activation(h_sb[:, r, :], x_sb[:, r, :],
                                             AF.Identity, bias=ab[:, r, 1:2],
                                             scale=ab[:, r, 0:1])
                    else:
                        # last tile: split halves across ScalarE/VectorE to
                        # shorten the critical path into the first qkv matmul
                        nc.scalar.activation(h_sb[:, r, 0:512],
                                             x_sb[:, r, 0:512],
                                             AF.Identity, bias=ab[:, r, 1:2],
                                             scale=ab[:, r, 0:1])
                        nc.vector.tensor_scalar(h_sb[:, r, 512:1024],
                                                x_sb[:, r, 512:1024],
                                                ab[:, r, 0:1], ab[:, r, 1:2],
                                                OP.mult, OP.add)


            gn_finish(0, CT - 1)
            gn_finish(CT - 1, CT)

        # ------------- qkv + attention (interleaved on PE) -------------
        # PSUM (8 banks): S chunks [128,2,512] x2 tags (4) + O pair-half
        # [128,2,512] (2) + background qkv/proj accumulators [128,512] x2 (2).
        with nc.named_scope("qkv_attn"), \
             tc.tile_pool(name="bgps", bufs=2, space="PSUM") as bgps, \
             tc.tile_pool(name="spool", bufs=1, space="PSUM") as spool, \
             tc.tile_pool(name="opool", bufs=1, space="PSUM") as opool, \
             tc.tile_pool(name="epool", bufs=3) as epool, \
             tc.tile_pool(name="rpool", bufs=2) as rpool, \
             tc.tile_pool(name="outp", bufs=4) as outp:

            def qk_task(dst, w_sb, b_sb, r, half, on_act=False):
                ps = bgps.tile([P, 512], FP32, tag="bgps",
                               name=f"qk_{r}_{half}_{w_sb.name}")
                for kc in range(CT):
                    nc.tensor.matmul(
                        ps, w_sb[:, kc, P * r:P * r + P],
                        h_sb[:, kc, 512 * half:512 * half + 512],
                        start=(kc == 0), stop=(kc == CT - 1))
                dsl = dst[:, r, 512 * half:512 * half + 512]
                if on_act:
                    nc.scalar.activation(dsl, ps, AF.Identity,
                                         bias=b_sb[:, r:r + 1], scale=1.0)
                else:
                    nc.vector.tensor_scalar(dsl, ps, b_sb[:, r:r + 1],
                                            None, OP.add)

            def vt_task(t):
                ps = bgps.tile([P, 512], FP32, tag="bgps", name=f"vt{t}")
                for kc in range(CT):
                    nc.tensor.matmul(ps, h_sb[:, kc, P * t:P * t + P],
                                     wv_sb[:, kc, :],
                                     start=(kc == 0), stop=(kc == CT - 1))
                nc.vector.tensor_copy(
                    vT_sb[:, t, :].rearrange("p (h c) -> p h c", c=128)[:, :, HD:128],
                    ps.rearrange("p (h c) -> p h c", c=HD))

            def xpb_task(rr, half):
                hs = 512 * half
                nc.vector.tensor_scalar(xpb_sb[:, rr, hs:hs + 512],
                                        x_sb[:, rr, hs:hs + 512],
                                        pb_sb[:, rr:rr + 1], None, OP.add)

            out_r = out.rearrange("(r p) n -> p r n", p=P)

            def proj_mms(ps, r, half, lo_pair):
                hs = 512 * half
                s = lo_pair
                if USE_FP8:
                    nc.tensor.matmul(
                        ps, pw_sb[:, 2 * s:2 * s + 2, P * r:P * r + P],
                        O_sb[:, 2 * s:2 * s + 2, hs:hs + 512],
                        start=True, stop=True, perf_mode=DR)
                else:
                    for kc in (2 * s, 2 * s + 1):
                        nc.tensor.matmul(
                            ps, pw_sb[:, kc, P * r:P * r + P],
                            O_sb[:, kc, hs:hs + 512],
                            start=(kc == 2 * s), stop=(kc == 2 * s + 1))

            def proj_part(r, half):
                # kc pair (0,1): heads 0..3 of this token half + x + pb,
                # accumulated in place into xpb
                hs = 512 * half
                ps = bgps.tile([P, 512], FP32, tag="bgps",
                               name=f"pp{r}_{half}")
                proj_mms(ps, r, half, 0)
                nc.vector.tensor_tensor(xpb_sb[:, r, hs:hs + 512], ps,
                                        xpb_sb[:, r, hs:hs + 512], OP.add)

            def proj_fin(r, half, eng_i=0):
                hs = 512 * half
                ps = bgps.tile([P, 512], FP32, tag="bgps",
                               name=f"pf{r}_{half}")
                proj_mms(ps, r, half, 1)
                o_sb = outp.tile([P, 512], FP32, tag="outsb",
                                 name=f"osb{r}_{half}")
                nc.vector.tensor_tensor(o_sb, ps,
                                        xpb_sb[:, r, hs:hs + 512], OP.add)
                eng = (nc.sync, nc.gpsimd)[eng_i % 2]
                eng.dma_start(out_r[:, r, hs:hs + 512], o_sb)

            # upfront: only what attention tile 0 needs (q0/k0 first halves)
            qk_task(q_sb, wq_sb, bq_sb, 0, 0, on_act=True)
            qk_task(k_sb, wk_sb, bk_sb, 0, 0, on_act=True)
            # second warm burst: fills the PE gap between the qkv upfront
            # matmuls and the first S tile so HAM never re-throttles
            wt2 = bgps.tile([P, 512], FP32, tag="bgps", name="warm2")
            for _ in range(8):
                nc.tensor.matmul(wt2, dummy[:, 0:128], dummy,
                                 start=True, stop=True)

            # drip key k fires right after stream tile T=k+1; keys 7,15,...
            # land just before a head-pair boundary where the PE otherwise
            # stalls on the last exp, so they hold PE-heavy tasks.
            drip = {
                0: [(qk_task, (q_sb, wq_sb, bq_sb, 1, 0))],
                1: [(qk_task, (k_sb, wk_sb, bk_sb, 1, 0))],
                2: [(vt_task, (0,)), (vt_task, (1,))],
                3: [(vt_task, (2,)), (vt_task, (3,))],
                4: [(vt_task, (4,))], 5: [(vt_task, (5,))],
                6: [(vt_task, (6,))], 7: [(vt_task, (7,))],
                10: [(qk_task, (q_sb, wq_sb, bq_sb, 2, 0))],
                11: [(qk_task, (k_sb, wk_sb, bk_sb, 2, 0))],
                13: [(xpb_task, (0, 0)), (xpb_task, (1, 0))],
                14: [(xpb_task, (2, 0)), (xpb_task, (3, 0))],
                15: [(qk_task, (q_sb, wq_sb, bq_sb, 3, 0))],
                16: [(qk_task, (k_sb, wk_sb, bk_sb, 3, 0))],
                20: [(proj_part, (0, 0))], 21: [(proj_part, (1, 0))],
                22: [(proj_part, (2, 0))], 23: [(proj_part, (3, 0))],
                26: [(qk_task, (q_sb, wq_sb, bq_sb, 0, 1))],
                27: [(qk_task, (k_sb, wk_sb, bk_sb, 0, 1))],
                32: [(proj_fin, (0, 0, 0))], 33: [(proj_fin, (1, 0, 1))],
                34: [(qk_task, (q_sb, wq_sb, bq_sb, 1, 1))],
                35: [(qk_task, (k_sb, wk_sb, bk_sb, 1, 1))],
                36: [(proj_fin, (2, 0, 0))], 37: [(proj_fin, (3, 0, 1))],
                39: [(qk_task, (q_sb, wq_sb, bq_sb, 2, 1))],
                40: [(qk_task, (k_sb, wk_sb, bk_sb, 2, 1))],
                42: [(xpb_task, (0, 1)), (xpb_task, (1, 1))],
                43: [(xpb_task, (2, 1)), (xpb_task, (3, 1))],
                46: [(qk_task, (q_sb, wq_sb, bq_sb, 3, 1))],
                47: [(qk_task, (k_sb, wk_sb, bk_sb, 3, 1))],
                52: [(proj_part, (0, 1))], 53: [(proj_part, (1, 1))],
                54: [(proj_part, (2, 1))], 55: [(proj_part, (3, 1))],
            }

            # ---- the unit stream: half-major, AV in DoubleRow m-tile pairs
            O_cur = [None]

            def emit_av_pair(pr, half, tp, E_sup, is_f8):
                for hi in range(2):
                    h8c = 128 * (2 * pr + hi)
                    if is_f8:
                        nc.tensor.matmul(
                            O_cur[0][:, hi, :],
                            vT_sb[:, 2 * tp:2 * tp + 2, h8c:h8c + 128],
                            E_sup[:, :, hi, :],
                            start=(tp == 0), stop=(tp == 3),
                            perf_mode=DR)
                    else:
                        for tpar in range(2):
                            nc.tensor.matmul(
                                O_cur[0][:, hi, :],
                                vT_sb[:, 2 * tp + tpar, h8c:h8c + 128],
                                E_sup[:, tpar, hi, :],
                                start=(tp == 0 and tpar == 0),
                                stop=(tp == 3 and tpar == 1))

            def emit_epilogue(pr, half, last=False):
                hs = 512 * half
                O_half = O_cur[0]
                Rh = rpool.tile([HD, 2, 512], FP32, tag="rh",
                                name=f"rh{pr}_{half}")
                if last:
                    # split per-hi so the final proj matmuls start sooner
                    for hi in range(2):
                        nc.vector.reciprocal_approx_fast(
                            Rh[:, hi, :], O_half[0:HD, hi, :])
                        nc.vector.tensor_tensor(
                            O_sb[HD * hi:HD * hi + HD, pr, hs:hs + 512],
                            O_half[HD:128, hi, :], Rh[:, hi, :], OP.mult)
                else:
                    nc.vector.reciprocal_approx_fast(Rh, O_half[0:HD, :, :])
                    for hi in range(2):
                        nc.vector.tensor_tensor(
                            O_sb[HD * hi:HD * hi + HD, pr, hs:hs + 512],
                            O_half[HD:128, hi, :], Rh[:, hi, :], OP.mult)

            pend = []  # deferred AV pair flushes: (tile_due, closure)
            T = 0
            fired = 0
            for half in range(2):
                for pr in range(4):
                    O_cur[0] = opool.tile([P, 2, 512], FP32, tag="oh",
                                          name=f"oh{pr}_{half}")
                    for tp in range(4):
                        gpi = (4 * half + pr) * 4 + tp
                        sch = gpi in SCH_PAIRS
                        is_f8 = USE_FP8 and not sch
                        if is_f8:
                            E_sup = epool.tile([P, 2, 2, 512], F8, tag="e8",
                                               name=f"e{gpi}")
                        else:
                            E_i = epool.tile([P, 2, 2, 512], I16, tag="ebf",
                                             name=f"e{gpi}")
                            E_sup = E_i.bitcast(BF16)
                        for tpar in range(2):
                            t = 2 * tp + tpar
                            stag = "s2a" if T % 2 == 0 else "s2b"
                            S_t = spool.tile([P, 2, 512], FP32, tag=stag,
                                             name=f"st{T}")
                            for hi in range(2):
                                nc.tensor.matmul(
                                    S_t[:, hi, :],
                                    k_sb[HD * hi:HD * hi + HD, pr,
                                         P * t:P * t + P],
                                    q_sb[HD * hi:HD * hi + HD, pr,
                                         512 * half:512 * half + 512],
                                    start=True, stop=True)
                            if sch:
                                nc.vector.tensor_scalar(
                                    E_i[:, tpar, :, :], S_t,
                                    A_SCH, B_SCH - SHIFT * A_SCH,
                                    OP.mult, OP.add)
                            elif USE_FP8:
                                nc.scalar.activation(E_sup[:, tpar, :, :],
                                                     S_t, AF.Exp,
                                                     bias=shift_sb,
                                                     scale=1.0)
                            else:
                                nc.scalar.activation(E_sup[:, tpar, :, :],
                                                     S_t, AF.Exp)
                            T += 1
                            # AV pairs flush with ~2-tile lag; drain hard
                            # near the stream end
                            lag = 3 if T < 57 else 0
                            while pend and pend[0][0] <= T - lag:
                                pend.pop(0)[1]()
                            for ci in range(fired, T):
                                for fn, args in drip.pop(ci, ()):
                                    fn(*args)
                            fired = T
                        pend.append((
                            T,
                            (lambda a=pr, b=half, c=tp, e=E_sup,
                             f=is_f8: emit_av_pair(a, b, c, e, f))))
                    while pend:
                        pend.pop(0)[1]()
                    emit_epilogue(pr, half, last=(half == 1 and pr == 3))
            assert not drip, drip

            # ---------------- tail: half-1 proj finals ----------------
            with nc.named_scope("proj"):
                for g in range(2):
                    ps2 = spool.tile([P, 2, 512], FP32,
                                     tag="s2a" if g == 0 else "s2b",
                                     name=f"pjt{g}")
                    for rr in range(2):
                        proj_mms(ps2[:, rr, :], 2 * g + rr, 1, 1)
                    o2 = outp.tile([P, 2, 512], FP32, tag="outsb2",
                                   name=f"osb2_{g}")
                    nc.vector.tensor_tensor(
                        o2, ps2, xpb_sb[:, 2 * g:2 * g + 2, 512:1024],
                        OP.add)
                    eng = nc.sync if g == 0 else nc.gpsimd
                    eng.dma_start(out_r[:, 2 * g:2 * g + 2, 512:1024], o2)

_CACHE: dict = {}


def _build():
    if "nc" in _CACHE:
        return _CACHE["nc"]
    nc = bacc.Bacc("TRN2", target_bir_lowering=False, debug=False,
                   num_devices=NCORES)
    wdt = F8 if USE_FP8 else BF16
    io = {
        "x": nc.dram_tensor("x", [C, NT], FP32, kind="ExternalInput").ap(),
        "wq": nc.dram_tensor("wq", [C, C], BF16, kind="ExternalInput").ap(),
        "wk": nc.dram_tensor("wk", [C, C], BF16, kind="ExternalInput").ap(),
        "wv": nc.dram_tensor("wv", [C, C], BF16, kind="ExternalInput").ap(),
        "pw": nc.dram_tensor("pw", [C, C], wdt, kind="ExternalInput").ap(),
        "smalls": nc.dram_tensor("smalls", [P, 28], FP32,
                                 kind="ExternalInput").ap(),
        "imat": nc.dram_tensor("imat", [NH, P], FP32,
                               kind="ExternalInput").ap(),
        "out": nc.dram_tensor("out", [C, NT], FP32, kind="ExternalOutput").ap(),
    }
    with tile.TileContext(nc) as tc:
        _emit(tc, io)
    nc.compile()
    _CACHE["nc"] = nc
    return nc


def _host_prep(inputs):
    x = np.ascontiguousarray(np.asarray(inputs["x"], dtype=np.float32))
    qkv_w = np.asarray(inputs["qkv_w"], dtype=np.float32)
    qkv_b = np.asarray(inputs["qkv_b"], dtype=np.float32)
    proj_w = np.asarray(inputs["proj_w"], dtype=np.float32)
    proj_b = np.asarray(inputs["proj_b"], dtype=np.float32)
    gn_scale = np.asarray(inputs["gn_scale"], dtype=np.float32)
    gn_bias = np.asarray(inputs["gn_bias"], dtype=np.float32)

    s = np.float32(1.0 / np.sqrt(HD))
    bf = ml_dtypes.bfloat16
    f8 = ml_dtypes.float8_e4m3 if USE_FP8 else bf

    def col(a):  # [(r p)] -> [p, r]
        return np.ascontiguousarray(a.reshape(CT, P).T)

    smalls = np.concatenate([
        col(gn_scale), col(gn_bias), col(qkv_b[0:C] * s), col(qkv_b[C:2 * C]),
        col((proj_b + proj_w @ qkv_b[2 * C:3 * C]).astype(np.float32)),
        # amat: [128, 8], 1/16 where channel p belongs to group j of its tile
        np.kron(np.eye(NH, dtype=np.float32),
                np.ones((GSZ, 1), np.float32)) / GSZ,
    ], axis=1).astype(np.float32)

    shared = {
        "wq": np.ascontiguousarray((qkv_w[0:C] * s).T).astype(bf),
        "wk": np.ascontiguousarray(qkv_w[C:2 * C].T).astype(bf),
        "wv": np.ascontiguousarray(qkv_w[2 * C:3 * C].T).astype(bf),
        "pw": np.ascontiguousarray(proj_w.T).astype(f8),
        "smalls": smalls,
        # imat: [8, 128], 1.0 where channel p belongs to group j of its tile
        "imat": np.ascontiguousarray(np.kron(np.eye(NH, dtype=np.float32),
                                             np.ones((1, GSZ), np.float32))),
    }
    B = x.shape[0]
    in_maps = []
    for b in range(B):
        m = dict(shared)
        m["x"] = np.ascontiguousarray(x[b].reshape(C, NT))
        in_maps.append(m)
    return in_maps


def run(inputs, trace=False):
    nc = _build()
    in_maps = _host_prep(inputs)
    res = run_bass_kernel_spmd(nc, in_maps, list(range(NCORES)), trace=trace)
    out = np.stack([res.results[i]["out"] for i in range(NCORES)], axis=0)
    return out.reshape(len(in_maps), C, 32, 32), res


def kernel(**inputs) -> np.ndarray:
    out, _ = run(inputs, trace=False)
    return out.astype(np.float32)
